# revision 10
# baseline (speedup 1.0000x reference)
"""Multi-head attention (B=2, S=2048, E=1024, H=16) on 8 Trainium2 NeuronCores.

Sharding: core c -> batch c//4, heads 4*(c%4)..4*(c%4)+3  (data + head parallel).
Each core computes a partial output projection [S, E] over its 256 head-dims;
the host sums the 4 partials per batch and adds the output bias.

v2 schedule (vs the v1 baseline at 178.7 us cost-model):
  * DMA stream order wq | xq | wk | xk | wv,wo,bias | xv with the Q/K
    projections e-outer, so the PE starts ~3 us in (v1 idled ~14 us waiting
    for the full xv stream before its first matmul).
  * Attention groups processed smallest-first (g0..g3); scores+exp for g0/g1
    are interleaved with the V projection (one 1-bank PSUM pass per s-tile,
    e-step-interleaved during the xv stream), so ACT starts ~30 us in.
  * Scores matmuls trimmed at the causal diagonal (PE -12K rows); PV trimmed
    too for groups with >=2 full-width key tiles (start/stop land on
    full-width matmuls; g0 falls back to full-width PV over zeroed lanes).
  * Zinv = vector.reciprocal(Z) on DVE (v1 burned ACT on exp(-ln Z)).
  * attnP = ev * bps with ev staged through SBUF on DVE (the ISA allows
    only one PSUM operand per DVE instruction).
  * Output partials stored bf16 (halves writeback DMA); host sums in f32.

Matmul operands are bf16 (full PE rate); accumulation is fp32 in PSUM.
Constant-shift softmax: probs = exp(s - 32), exact after the Zinv multiply
up to fp32 rounding; masked lanes are zeroed by a 0/1 bf16 multiply after
exp (partial diagonal blocks) or simply never read (trimmed lanes).
"""

import sys

for _p in ("/opt/trn_rl_repo", "/root/.axon_site/_ro/trn_rl_repo"):
    if _p not in sys.path:
        sys.path.insert(0, _p)

import numpy as np


# ---------------------------------------------------------------------------
# Patch: the walrus build in this container rejects >1 sem wait on one CTRL
# instruction and the TileContext exit drain aggregates every outstanding
# proc's wait onto a single Drain. Spill the excess waits onto SP nops.
# ---------------------------------------------------------------------------
def _install_tile_drain_patch():
    import concourse.tile as tile
    import concourse.mybir as mybir
    from concourse.vector_clock import ScopedClock

    if getattr(tile.TileContext, "_drain_patch_installed", False):
        return

    def _patched_drain_and_barrier(self, tick_clock, wait_clock):
        drain_inst = self.nc.sync.drain()
        wait_clock.add_sem_waits(
            drain_inst.ins, ScopedClock({None: tick_clock.global_clock})
        )
        si = drain_inst.ins.sync_info
        waits = list(si.on_wait) if si and si.on_wait else []
        if len(waits) > 1:
            si.on_wait = waits[:1]
            for w in waits[1:]:
                nop = self.nc.sync.nop(nofuse=True, hint="drain_wait_spill")
                nop.ins.sync_info = mybir.SyncInfo(on_wait=[w], on_update=[])
        self.nc.all_engine_barrier()
        assert self.sems is not None
        popped = self.nc._tile_sem_poison_stack.pop()
        assert popped is self._sem_poison
        self.nc.clear_and_free_semaphores(list(self.sems.allocated().values()))

    tile.TileContext._drain_and_barrier = _patched_drain_and_barrier
    tile.TileContext._drain_patch_installed = True


def _split_multi_waits(nc, maxw=1):
    """Hoist excess sem waits onto engine-queue NoOps inserted just before
    the instruction (sequencer executes them in order; semantics identical)."""
    import concourse.mybir as mybir

    ctr = 0
    for bb in nc.main_func.blocks:
        new = []
        for inst in bb.instructions:
            si = inst.sync_info
            waits = list(si.on_wait) if si and si.on_wait else []
            if len(waits) > maxw:
                extras = waits[:-maxw]
                si.on_wait = waits[-maxw:]
                for i in range(0, len(extras), maxw):
                    nop = mybir.InstNoOp(
                        name=f"I-waitspill-{ctr}", engine=inst.engine,
                        ins=[], outs=[])
                    ctr += 1
                    nop.sync_info = mybir.SyncInfo(
                        on_wait=extras[i:i + maxw], on_update=[])
                    try:
                        nc.register_instruction(nop, overwrite=True)
                    except Exception:
                        pass
                    new.append(nop)
            new.append(inst)
        bb.instructions = new


# ---------------------------------------------------------------------------
# Mask classification (host side, from the actual mask array).
# Blocks are 128x128 in the transposed score layout: block (kt, qb) covers
# keys kt*128.. x queries qb*128..
# ---------------------------------------------------------------------------
def classify_mask(mask2d, S, KB=128):
    nb = S // KB
    assert mask2d.shape == (S, S)
    assert mask2d.any(axis=1).all(), "a query row with no attended key"
    maskT = mask2d.T  # [keys, q]
    uniq = {}
    biases = []
    bias_idx = {}  # (kt, qb) -> None (all attended) or index
    block_live = np.zeros((nb, nb), dtype=bool)
    for kt in range(nb):
        for qb in range(nb):
            blk = maskT[kt * KB:(kt + 1) * KB, qb * KB:(qb + 1) * KB]
            if blk.all():
                bias_idx[(kt, qb)] = None
                block_live[kt, qb] = True
            else:
                b = np.where(blk, np.float32(1.0), np.float32(0.0))
                key = b.tobytes()
                if key not in uniq:
                    uniq[key] = len(biases)
                    biases.append(b)
                bias_idx[(kt, qb)] = uniq[key]
                block_live[kt, qb] = blk.any()
    return bias_idx, biases, block_live


# ---------------------------------------------------------------------------
# Bass program builder (one SPMD program, same for all cores).
# ---------------------------------------------------------------------------
def build_nc(S, E, D, HL, bias_idx, block_live, nuniq, shift=32.0, repeat=1):
    import concourse.bass as bass
    import concourse.mybir as mybir
    import concourse.tile as tile

    f32 = mybir.dt.float32
    bf16 = mybir.dt.bfloat16
    Act = mybir.ActivationFunctionType

    P = 128
    EC = E // P              # E chunks (contraction tiles for projections)
    DIM = HL * D             # this core's head dims (256)
    MT = DIM // P            # m-tiles of QT/KT (2)
    QG = 512                 # q-group width
    NQG = S // QG
    NKT = S // P             # key tiles
    NST = S // P             # s tiles
    QB = QG // P             # q blocks per group (4)
    VW = HL * (D + 1)        # V width incl. ones columns (260)
    EGW = min(QG, E)         # output E slice width
    NEG = E // EGW           # output E slices (2)

    # ---- per-(g, kt) live q-window (trim at the causal diagonal) ----
    def span(g, kt):
        live = [j for j in range(QB) if block_live[kt, g * QB + j]]
        if not live:
            return None
        lo, hi = min(live), max(live)
        if hi != QB - 1:
            # non-suffix window (non-causal mask): fall back to full width;
            # dead blocks are zeroed via the bias-mask path
            return 0, QG
        return lo * P, (hi - lo + 1) * P

    def kts_for_group(g):
        return [kt for kt in range(NKT) if span(g, kt) is not None]

    nc = bass.Bass()
    dp = nc.declare_dram_parameter
    d_xq = dp("xqT", [E, S], bf16, isOutput=False)
    d_xk = dp("xkT", [E, S], bf16, isOutput=False)
    d_xv = dp("xvT", [E, S], bf16, isOutput=False)
    d_wq = dp("wq", [E, DIM], bf16, isOutput=False)
    d_wk = dp("wk", [E, DIM], bf16, isOutput=False)
    d_wv = dp("wv", [E, VW], bf16, isOutput=False)
    d_wo = dp("wo", [DIM, E], bf16, isOutput=False)
    d_bias = dp("biasT", [P, max(nuniq, 1) * P], bf16, isOutput=False)
    d_out = dp("out_p", [S, E], bf16, isOutput=True)

    import contextlib
    with tile.TileContext(nc) as tc, contextlib.ExitStack() as _stk:
        consts = _stk.enter_context(tc.tile_pool(name="consts", bufs=1))

        w_sb = {}
        for nm, width in (("wq", DIM), ("wk", DIM), ("wv", VW)):
            w_sb[nm] = consts.tile([P, EC, width], bf16, name=f"sb_{nm}",
                                   tag=f"sb_{nm}")
        w_dram = {"wq": d_wq, "wk": d_wk, "wv": d_wv}
        wo_sb = [consts.tile([2 * D, E], bf16, name=f"sb_wo{p}",
                             tag=f"sb_wo{p}") for p in range(HL // 2)]
        bias_sb = consts.tile([P, max(nuniq, 1) * P], bf16, name="sb_bias")
        ones128 = consts.tile([P, D], bf16, name="ones128")
        nc.vector.memset(ones128, 1.0)
        # [65, 128] selector for the pair-broadcast matmul: row 0 routes
        # head 2p's zinv to partitions 0..63, row 64 routes head 2p+1's to
        # 64..127; zero rows 1..63 annihilate the untouched zpair rows.
        ones65 = consts.tile([D + 1, 2 * D], bf16, name="ones65")
        nc.vector.memset(ones65, 0.0)
        nc.vector.memset(ones65[0:1, 0:D], 1.0)
        nc.vector.memset(ones65[D:D + 1, D:2 * D], 1.0)
        negshift = consts.tile([P, 1], f32, name="negshift")
        nc.vector.memset(negshift, -shift)

        def load_w(nm, e=None):
            if e is None:
                nc.sync.dma_start(
                    out=w_sb[nm],
                    in_=w_dram[nm][:, :].rearrange("(e p) n -> p e n", p=P))
            else:
                nc.sync.dma_start(
                    out=w_sb[nm][:, e, :],
                    in_=w_dram[nm][e * P:(e + 1) * P, :])

        def emit_once():
            # persistent projection outputs
            QT = [consts.tile([P, S], bf16, name=f"QT{m}", tag=f"QT{m}")
                  for m in range(MT)]
            KT = [consts.tile([P, S], bf16, name=f"KT{m}", tag=f"KT{m}")
                  for m in range(MT)]
            V = [consts.tile([P, VW], bf16, name=f"V{s}", tag=f"V{s}")
                 for s in range(NST)]
            attnP = [[consts.tile([2 * D, QG], bf16, name=f"attnP{p}g{g}",
                                  tag=f"attnP{p}g{g}") for g in range(NQG)]
                     for p in range(HL // 2)]
            # zinv staging, one [65, QG] tile per (group, head-pair): rows
            # 0/64 get the two heads' reciprocals, rows 1..63 stay zero.
            zpairs = {}
            for g in range(NQG):
                for p in range(HL // 2):
                    zp = consts.tile([D + 1, QG], bf16, name=f"zp{g}{p}",
                                     tag=f"zp{g}{p}")
                    nc.gpsimd.memset(zp, 0.0)
                    zpairs[(g, p)] = zp

            xt_pool = _stk.enter_context(
                tc.tile_pool(name="xt", bufs=EC + 4))

            def stream_chunks(dram, split_last=False):
                chunks = []
                for e in range(EC):
                    ch = xt_pool.tile([P, S], bf16, tag="xt", name=f"xch{e}")
                    if split_last and e == EC - 1:
                        # halve the final chunk: the m0 pass's last matmuls
                        # (the global schedule anchor) start ~0.7us earlier
                        nc.sync.dma_start(
                            out=ch[:, 0:S // 2],
                            in_=dram[e * P:(e + 1) * P, 0:S // 2])
                        nc.sync.dma_start(
                            out=ch[:, S // 2:S],
                            in_=dram[e * P:(e + 1) * P, S // 2:S])
                    else:
                        nc.sync.dma_start(out=ch,
                                          in_=dram[e * P:(e + 1) * P, :])
                    chunks.append(ch)
                return chunks

            # Bank plan (8 PSUM banks):
            #   {B} = 4 banks: psS (2 x [128,1024] scores tiles), open the
            #         whole kernel -- never overlaps psA, so the first
            #         scores matmul has no pool-release dependency.
            #   {A} = 4 banks: psA (4 x [128,512] projection tiles, m-tile
            #         half-passes) -> psV(2) + psPV(2) -> psPV(2) + psB(1)
            #         + psO(2).
            pools = {}
            probs_pool = _stk.enter_context(
                tc.tile_pool(name="probs", bufs=38))
            z_pool = _stk.enter_context(tc.tile_pool(name="zrow", bufs=4))
            ev_pool = _stk.enter_context(tc.tile_pool(name="ev", bufs=4))
            out_pool = _stk.enter_context(tc.tile_pool(name="outst", bufs=6))
            psS_cm = tc.tile_pool(name="psS", bufs=2, space="PSUM")
            psS = psS_cm.__enter__()

            # ---- attention slot units (defined early; emitted below) ----
            slot_pb = {}      # (g,h) -> list of (pair, pbtile)

            def scores_unit(g, h, pair):
                m, po = h // 2, (h % 2) * D
                sps = psS.tile([P, 2 * QG], f32, tag="s")
                pb = probs_pool.tile([P, 2 * QG], bf16, tag="pb")
                for x, kt in enumerate(pair):
                    qoff, w = span(g, kt)
                    nc.tensor.matmul(
                        sps[:, x * QG + qoff:x * QG + qoff + w],
                        lhsT=KT[m][po:po + D, kt * P:(kt + 1) * P],
                        rhs=QT[m][po:po + D,
                                  g * QG + qoff:g * QG + qoff + w],
                        start=True, stop=True)
                dead = sum(QG - span(g, kt)[1] for kt in pair)
                wtot = len(pair) * QG
                full_read = (g == 0)
                if not full_read and dead >= 512 and len(pair) == 2:
                    # trimmed exps for the trailing diagonal pair
                    for x, kt in enumerate(pair):
                        qoff, w = span(g, kt)
                        nc.scalar.activation(
                            pb[:, x * QG + qoff:x * QG + qoff + w],
                            sps[:, x * QG + qoff:x * QG + qoff + w],
                            Act.Exp, bias=negshift[:, 0:1])
                else:
                    nc.scalar.activation(pb[:, 0:wtot], sps[:, 0:wtot],
                                         Act.Exp, bias=negshift[:, 0:1])
                if full_read:
                    # g0's dead lanes are read by full-width PV: zero them
                    for x, kt in enumerate(pair):
                        qoff, w = span(g, kt)
                        if qoff > 0:
                            nc.gpsimd.memset(
                                pb[:, x * QG:x * QG + qoff], 0.0)
                for x, kt in enumerate(pair):
                    qoff, w = span(g, kt)
                    for j in range(qoff // P, (qoff + w) // P):
                        qb = g * QB + j
                        bidx = bias_idx[(kt, qb)]
                        if bidx is None:
                            continue
                        blkslice = pb[:, x * QG + j * P:
                                      x * QG + (j + 1) * P]
                        if not block_live[kt, qb]:
                            nc.gpsimd.memset(blkslice, 0.0)
                        else:
                            # gpsimd, not DVE: frees the vector engine for
                            # the ev/recip chain the pv ring depends on
                            nc.gpsimd.tensor_mul(
                                blkslice, blkslice,
                                bias_sb[:, bidx * P:(bidx + 1) * P])
                slot_pb[(g, h)].append((pair, pb))

            def scores_units(g, h):
                slot_pb[(g, h)] = []
                kts = kts_for_group(g)
                pairs = [kts[i:i + 2] for i in range(0, len(kts), 2)]
                return [(lambda g=g, h=h, pair=pair:
                         scores_unit(g, h, pair)) for pair in pairs]

            evs = {}

            def pbmap_get(g, h, kt):
                return pv_unit._pbmap[kt]

            def pv_unit(g, h):
                """PV accumulation matmuls; immediately stages zinv (DVE
                reciprocal into the pair tile) and ev (SBUF copy), so the
                psum bank frees without waiting for the broadcast."""
                kts = kts_for_group(g)
                full_read = (g == 0)
                pv = pools["psPV"].tile([D + 1, QG], f32, tag="pv")
                pbmap = {}
                for pair, pb in slot_pb.pop((g, h)):
                    for x, kt in enumerate(pair):
                        pbmap[kt] = (pb, x * QG)
                pv_unit._pbmap = pbmap
                fulls = [kt for kt in kts
                         if full_read or span(g, kt) == (0, QG)]
                partials = [kt for kt in kts if kt not in fulls]
                assert len(fulls) >= 2 or not partials, (g, fulls)
                # single stop on the final full-width matmul: the tile
                # framework releases readers on stop, so every column must
                # be final by then (multi-stop per tile races)
                order = fulls[:1] + partials + fulls[1:]
                for i, kt in enumerate(order):
                    pb, xoff = pbmap_get(g, h, kt)
                    qoff, w = (0, QG) if full_read else span(g, kt)
                    nc.tensor.matmul(
                        pv[:, qoff:qoff + w],
                        lhsT=V[kt][:, h * (D + 1):(h + 1) * (D + 1)],
                        rhs=pb[:, xoff + qoff:xoff + qoff + w],
                        start=(i == 0), stop=(i == len(order) - 1))
                po = (h % 2) * D
                zp = zpairs[(g, h // 2)]
                with nc.allow_low_precision("zinv bf16 bcast operand"):
                    nc.vector.reciprocal(zp[po:po + 1, :], pv[D:D + 1, :])
                ev = ev_pool.tile([D, QG], f32, tag="ev", name="ev")
                nc.vector.tensor_copy(ev, pv[0:D, :])
                evs[(g, h)] = ev

            def bm_pair(g, p):
                """One K=65 matmul broadcasts both heads' zinv rows across
                the partition dim, then the two attnP multiplies."""
                bps = pools["psB"].tile([P, QG], f32, tag="b")
                nc.tensor.matmul(
                    bps, lhsT=ones65, rhs=zpairs[(g, p)],
                    start=True, stop=True)
                for x in range(2):
                    po = x * D
                    nc.vector.tensor_mul(
                        attnP[p][g][po:po + D, :],
                        evs.pop((g, 2 * p + x)), bps[po:po + D, :])

            def oproj_st(g, j, tail=False, last=False):
                st = g * QB + j
                off = j * P
                ot = out_pool.tile([P, NEG * EGW], bf16, tag="ot")
                for eg in range(NEG):
                    ops = pools["psO"].tile([P, EGW], f32, tag="o",
                                            name="opso")
                    for p in range(HL // 2):
                        nc.tensor.matmul(
                            ops,
                            lhsT=attnP[p][g][:, off:off + P],
                            rhs=wo_sb[p][:, eg * EGW:(eg + 1) * EGW],
                            start=(p == 0), stop=(p == HL // 2 - 1))
                    ots = ot[:, eg * EGW:(eg + 1) * EGW]
                    if eg % 2 == 0:
                        # scalar engine: mid-stream its exp queue has slack
                        # and exps are not the anchor; tail-time it is idle
                        nc.scalar.copy(ots, ops)
                    else:
                        nc.vector.tensor_copy(ots, ops)
                    if last:
                        # final s-tile: per-eg DMA so the writeback
                        # overlaps the remaining copy
                        nc.sync.dma_start(
                            out=d_out[st * P:(st + 1) * P,
                                      eg * EGW:(eg + 1) * EGW], in_=ots)
                if not last:
                    nc.sync.dma_start(
                        out=d_out[st * P:(st + 1) * P, :], in_=ot)

            def group_units(g, with_oproj=True):
                """pv units (which stage zinv/ev on DVE as they finish),
                one pair-broadcast + attnP-mul unit per head pair, then
                optionally the group's output projection."""
                us = [lambda g=g: pv_unit(g, 0),
                      lambda g=g: pv_unit(g, 1),
                      lambda g=g: bm_pair(g, 0),
                      lambda g=g: pv_unit(g, 2),
                      lambda g=g: pv_unit(g, 3),
                      lambda g=g: bm_pair(g, 1)]
                if with_oproj:
                    for j in range(QB):
                        us.append(lambda g=g, j=j: oproj_st(g, j))
                return us

            # ---------------- phase A: projections --------------------
            # psA: 4 banks, one [128,512] tile per q-group; each weight
            # matrix runs as two m-tile half-passes over the SBUF-resident
            # chunks (the second pass re-walks them, matmul cost is
            # free-size-only so this is free on the PE).
            def qk_pass(wname, dst, m, chunks, psA, weave=None):
                pss = [psA.tile([P, QG], f32, tag=f"psA{g}",
                                name=f"psqk{g}") for g in range(NQG)]
                for e in range(EC):
                    for g in range(NQG):
                        nc.tensor.matmul(
                            pss[g],
                            lhsT=w_sb[wname][:, e, m * P:(m + 1) * P],
                            rhs=chunks[e][:, g * QG:(g + 1) * QG],
                            start=(e == 0), stop=(e == EC - 1))
                    if weave is not None and weave:
                        weave.pop(0)()
                for g in range(NQG):
                    nc.vector.tensor_copy(
                        dst[m][:, g * QG:(g + 1) * QG], pss[g])

            with tc.tile_pool(name="psA", bufs=1, space="PSUM") as psA:
                load_w("wq")
                qchunks = stream_chunks(d_xq)
                load_w("wk")
                qk_pass("wq", QT, 0, qchunks, psA)
                qk_pass("wq", QT, 1, qchunks, psA)
                kchunks = stream_chunks(d_xk, split_last=True)
                load_w("wv")
                for p in range(HL // 2):
                    nc.sync.dma_start(
                        out=wo_sb[p], in_=d_wo[p * 2 * D:(p + 1) * 2 * D, :])
                nc.sync.dma_start(out=bias_sb, in_=d_bias[:, :])
                vchunks = stream_chunks(d_xv)
                qk_pass("wk", KT, 0, kchunks, psA)
                # weave the first m0 scores between KT m1 e-steps: the PE
                # reaches them as soon as the KT m0 copies land.
                weave = (scores_units(0, 0) + scores_units(0, 1)
                         + scores_units(1, 0))
                qk_pass("wk", KT, 1, kchunks, psA, weave=weave)
                for u in weave:
                    u()

            # ---------------- attention ------------------------------
            # V projection: one s-tile per 1-bank pass; st0's chunk-gated
            # e-steps are paced between score units so they never
            # head-block the PE queue.
            vstate = {"st": 0, "e": 0, "ps": None}

            def v_unit():
                st = vstate["st"]
                if st >= NST:
                    return False
                e = vstate["e"]
                if e == 0:
                    vstate["ps"] = pools["psV"].tile(
                        [P, VW], f32, tag="psv", name="psv")
                ps = vstate["ps"]
                nc.tensor.matmul(
                    ps,
                    lhsT=vchunks[e][:, st * P:(st + 1) * P],
                    rhs=w_sb["wv"][:, e, :],
                    start=(e == 0), stop=(e == EC - 1))
                if e == EC - 1:
                    nc.vector.tensor_copy(V[st], ps)
                    onescols = V[st].rearrange(
                        "p (h c) -> p h c", c=D + 1)[:, :, D]
                    nc.gpsimd.memset(onescols, 1.0)
                    vstate["st"] = st + 1
                    vstate["e"] = 0
                else:
                    vstate["e"] = e + 1
                return True

            with tc.tile_pool(name="psPV", bufs=2, space="PSUM") as psPV:
                pools["psPV"] = psPV
                with tc.tile_pool(name="psV", bufs=2, space="PSUM") as psV:
                    pools["psV"] = psV
                    # remaining m0/m1 scores with v(st0..) paced through
                    mid = (scores_units(0, 2) + scores_units(0, 3)
                           + scores_units(1, 1) + scores_units(1, 2)
                           + scores_units(1, 3))
                    # no V units during mid: psV's banks are psA's, whose
                    # release (last KT copies) lands ~41.5us; emitting V
                    # earlier just head-blocks the PE queue on the release
                    for u in mid:
                        u()
                    # pace the remaining chunk-gated V e-steps through the
                    # g2 scores, then burst the rest (chunks all resident).
                    g2s = []
                    for h in range(HL):
                        g2s.extend(scores_units(2, h))
                    for i, u in enumerate(g2s):
                        u()
                        for _ in range(4):
                            v_unit()
                        # g0's pv units as ring-throttle filler once V
                        # s-tiles 0-3 are projected (psPV is already open)
                        if i in (9, 11, 13, 15):
                            pv_unit(0, (i - 9) // 2)
                    while v_unit():
                        pass

                with tc.tile_pool(name="psB", bufs=1, space="PSUM") as psB, \
                     tc.tile_pool(name="psO", bufs=1, space="PSUM") as psO:
                    pools["psB"] = psB
                    pools["psO"] = psO
                    g3scores = []
                    for h in range(HL):
                        g3scores.extend(scores_units(3, h))
                    # PE-cost-paced weave of g0/g1/g2 pv-fin-oproj units
                    # with g3 scores: one scores unit (~1.9us of ACT food)
                    # per ~1.6us of PE work keeps ACT from running dry.
                    def a_cost(g, with_oproj=True):
                        rows = sum((span(g, kt)[1] if g else QG)
                                   for kt in kts_for_group(g))
                        pvu = rows * 0.42e-3
                        return ([pvu, pvu, 0.25, pvu, pvu, 0.25]
                                + ([0.95] * QB if with_oproj else []))
                    # g0's pvs already ran as g2s filler: only its two
                    # broadcast/mul pairs and oproj remain
                    g0_rest = [lambda: bm_pair(0, 0), lambda: bm_pair(0, 1)]
                    for j in range(QB):
                        g0_rest.append(lambda j=j: oproj_st(0, j))
                    stream_a = (g0_rest + group_units(1)
                                + group_units(2, with_oproj=False))
                    costs = ([0.25, 0.25] + [0.95] * QB
                             + a_cost(1) + a_cost(2, False))
                    wa, wb = [], list(g3scores)
                    acc = 0.0
                    for i, u in enumerate(stream_a):
                        wa.append(u)
                        acc += costs[i]
                        while acc >= 0.7 and wb:
                            wa.append(wb.pop(0))
                            acc -= 0.7
                    wa.extend(wb)
                    # g3's first three heads while the old pools are open
                    wa.append(lambda: pv_unit(3, 0))
                    wa.append(lambda: pv_unit(3, 1))
                    wa.append(lambda: bm_pair(3, 0))
                    wa.append(lambda: pv_unit(3, 2))
                    for u in wa:
                        u()

            # Close every psum pool (LIFO); the tail re-opens fresh
            # double-buffered banks and fills the zinv/broadcast latency of
            # g3's last head with g2's deferred output projection.
            psS_cm.__exit__(None, None, None)
            with tc.tile_pool(name="psPV2", bufs=2, space="PSUM") as psPV2, \
                 tc.tile_pool(name="psB2", bufs=2, space="PSUM") as psB2, \
                 tc.tile_pool(name="psO2", bufs=4, space="PSUM") as psO2:
                pools["psPV"] = psPV2
                pools["psB"] = psB2
                pools["psO"] = psO2
                pv_unit(3, 3)
                oproj_st(2, 0, tail=True)
                bm_pair(3, 1)
                for j in range(1, QB):
                    oproj_st(2, j, tail=True)
                for j in range(QB):
                    oproj_st(3, j, tail=True, last=(j == QB - 1))

        for _rep in range(repeat):
            emit_once()

    _split_multi_waits(nc)
    return nc


# ---------------------------------------------------------------------------
# Host entry point
# ---------------------------------------------------------------------------
LAST_EXEC_NS = None
LAST_RESULT = None


def kernel(query, key, value, mask, Wq, Wk, Wv, Wo, bo):
    global LAST_EXEC_NS, LAST_RESULT
    _install_tile_drain_patch()
    from concourse.bass_utils import run_bass_kernel_spmd

    B, S, E = 2, 2048, 1024
    H, D = 16, 64
    N_CORES = 8
    BG = 2                    # batch groups
    HG = N_CORES // BG        # head groups per batch
    HL = H // HG              # heads per core
    DIM = HL * D

    query = np.asarray(query, dtype=np.float32)
    key = np.asarray(key, dtype=np.float32)
    value = np.asarray(value, dtype=np.float32)
    mask2d = np.asarray(mask).reshape(S, S).astype(bool)
    Wq = np.asarray(Wq, dtype=np.float32)
    Wk = np.asarray(Wk, dtype=np.float32)
    Wv = np.asarray(Wv, dtype=np.float32)
    Wo = np.asarray(Wo, dtype=np.float32)
    bo = np.asarray(bo, dtype=np.float32)

    bias_idx, biases, block_live = classify_mask(mask2d, S)
    nuniq = len(biases)
    bias_stack = (np.concatenate(biases, axis=1) if nuniq
                  else np.zeros((128, 128), np.float32))

    nc = build_nc(S, E, D, HL, bias_idx, block_live, nuniq)

    scale = np.float32(1.0 / np.sqrt(D))
    in_maps = []
    for c in range(N_CORES):
        b, hg = c // HG, c % HG
        cols = slice(hg * DIM, (hg + 1) * DIM)
        wv_l = Wv[:, cols].reshape(E, HL, D)
        wv_aug = np.zeros((E, HL, D + 1), np.float32)
        wv_aug[:, :, :D] = wv_l
        in_maps.append({
            "xqT": _bf16(query[b].T),
            "xkT": _bf16(key[b].T),
            "xvT": _bf16(value[b].T),
            "wq": _bf16(Wq[:, cols] * scale),
            "wk": _bf16(Wk[:, cols]),
            "wv": _bf16(wv_aug.reshape(E, HL * (D + 1))),
            "wo": _bf16(Wo[cols, :]),
            "biasT": _bf16(bias_stack),
        })

    res = run_bass_kernel_spmd(nc, in_maps, list(range(N_CORES)))
    LAST_RESULT = res
    LAST_EXEC_NS = res.exec_time_ns or res.mean_exec_time_ns

    out = np.empty((B, S, E), np.float32)
    for b in range(BG):
        acc = res.results[b * HG]["out_p"].astype(np.float32)
        for j in range(1, HG):
            acc = acc + res.results[b * HG + j]["out_p"].astype(np.float32)
        out[b] = acc + bo[None, :]
    return out


def _bf16(a):
    import ml_dtypes
    return np.ascontiguousarray(np.asarray(a, np.float32)).astype(
        ml_dtypes.bfloat16)
